# revision 48
# baseline (speedup 1.0000x reference)
"""Trainium2 Bass kernel for BrainFunctionalConnectivityFeatureExtractionModule.

Math (per batch b, all f32):
    w    = relu(adj + adj_bias)                       (16,16)
    d    = 1/sqrt(sum(w, axis=1) + 1e-5)              (16,)
    lap  = I - d[:,None] * w * d[None,:]              (16,16)
    t1   = lap @ x[b]                                 (16,256)
    cp   = interleave(ones, t1)                       (16,512)
    h    = relu(brelu_bias + cp @ cheb_w)             (16,64)
    out  = h @ fc_w.T + fc_b                          (16,387)

Since the even interleaved lanes of cp are all-ones,
    cp @ cheb_w = t1 @ cheb_w[1::2] + sum(cheb_w[0::2], axis=0)
and since the (per-graph) node mix `lap` commutes with the channel
contraction W1 = cheb_w[1::2], the module collapses to
    y   = x @ W1                                       (rows,64)
    h   = relu(mix_lap(y) + bias_h)                    (rows,64)
    out = h @ fc_w.T + fc_b                            (rows,387)

Device mapping: pure data parallel over 8 cores, B=8192 -> 1024 batches/core,
ROWS = 1024*16 = 16384 (b,e)-rows per core, processed in 1024-row macro
tiles of 8 x 128-row sub-tiles (sub-tile = 8 full 16-node graphs); the big
tile halves per-tile fixed costs and group-switch bubbles vs 512-row tiles.

x is pre-transposed per macro tile ON THE HOST to xT [C, 1024 rows] (2KB
contiguous bf16 runs per partition -- ideal DMA shape), so on-chip:
  y_sub[128 n, 64]  = xT_chunk_slice^T @ W1_chunk     (16 mm x 64 cols, PSUM-acc)
  hT[64, n-slice]   = y_sub^T @ (I_8 (x) lap^T)        (8 mm x 128 cols)
  hT_sb             = relu(hT + bias_h); ones row appended (fc_b fold)
  out[128, 388pad]  = hT_slice^T @ fc_wT               (8 mm x 388 cols)
PE stream: 2576 cols per 512 rows (vs 3600 for the lap-first ordering).

Measured HW behavior this kernel is shaped around (from NTFF profiles):
 - The PE boots in the 1.2 GHz mid p-state and only reaches 2.4 GHz after
   a ~3.5us+ stretch of dense matmul streaming.  A ~4us warm-up burst of
   back-to-back matmuls (on a zeroed tile, while x(0) loads) releases the
   clock before real work starts; if the ramp stretch is too short the
   clock freezes partially ramped (~64ns y-stride instead of 53, ~18%
   slower end-to-end).  The threshold is environment-dependent (chip
   thermal state): NWU=10 ramped fully in some runs and only partially in
   others, so NWU=14 buys margin for ~0.9us of extra startup.
 - ~3us after the ramp, the HW activity monitor clamps utilization to 4/8
   for the rest of the run: matmul STREAMING then runs at an effective
   0.83ns/col while LDWEIGHTS still runs at full 2.4 GHz.  PE columns are
   therefore the scarce resource (hence the lap-last ordering), and the
   h-bias rides in the scalar activation, not in a K=1 bias matmul.
 - Iteration t runs mix(t-1), y(t), stage3(t-2) on the tensor engine, so
   every segment consumes inputs finished a full iteration earlier.
 - Engine assignment keeps the stage-3 gate chain short: scalar does only
   the relu+bias activation and the small y copy (both early-iteration
   deps); vector drains all four o_ps tiles; o-stores issue from the
   scalar HWDGE ring so they never head-of-line-block x prefetches on the
   SP ring (x is prefetched 2 tiles ahead, 6 buffers deep).
 - hT rotates through 3 long-lived buffers whose constant ones row (the
   fc_b fold) is written once at startup.
 - Epilogue: the last two tiles split their PSUM drains across both
   engines and store in halves on separate DGE rings to shorten the tail
   (storing quarters regressed: DGE generation overhead beats the gain).

All matmul inputs are bf16 (PSUM accumulation is f32): fp32/fp32r matmuls
hit a 2-4x slower datapath on trn2.  I/O is bf16 end-to-end: x is cast
(and tile-transposed) on the host, the output is stored bf16 and upcast on
the host.  rel-err gate is 2e-2; bf16 I/O lands ~3.6e-3.  HBM traffic/core:
8.4MB in + 12.7MB out = 21MB (~60us at the 358GB/s per-core DMA roofline).
Measured full-scale (nt=16 x 1024-row tiles, 8 cores): ~96-98us vs
195us baseline; steady state is PE-issue-bound at the duty cap with zero
measured tensor-engine idle.
"""

import numpy as np
from contextlib import ExitStack

B, E, C, H, OUT = 8192, 16, 256, 64, 387
NCORES = 8
ROWS = (B // NCORES) * E        # 16384 rows per core
NS = 8                          # sub-tiles per macro tile
TR = 128 * NS                   # 512 macro-tile rows
NT = ROWS // TR                 # 32 macro tiles per core
KC = C // 128                   # 2 contraction chunks of 128
OUTP = OUT + 1                  # fc matmul N padded even

_cache = {}


def _build_module(nt=NT):
    import concourse.tile as tile
    from concourse import bacc, mybir

    f32 = mybir.dt.float32
    bf16 = mybir.dt.bfloat16
    Relu = mybir.ActivationFunctionType.Relu

    rows = nt * TR
    nc = bacc.Bacc("TRN2", target_bir_lowering=False, debug=False,
                   num_devices=NCORES)

    # x arrives tile-transposed: [nt, C, 512]
    x_d = nc.dram_tensor("x", (nt, C, TR), bf16, kind="ExternalInput").ap()
    r_d = nc.dram_tensor("r", (128, 128), bf16, kind="ExternalInput").ap()
    w1_d = nc.dram_tensor("w1", (KC, 128, H), bf16, kind="ExternalInput").ap()
    bh_d = nc.dram_tensor("bh", (H, 1), f32, kind="ExternalInput").ap()
    fcw_d = nc.dram_tensor("fcw", (H + 1, OUTP), bf16, kind="ExternalInput").ap()
    o_d = nc.dram_tensor("o", (rows, OUT), bf16, kind="ExternalOutput").ap()

    with tile.TileContext(nc) as tc:
        with ExitStack() as ctx:
            consts = ctx.enter_context(tc.tile_pool(name="consts", bufs=1))
            xp = ctx.enter_context(tc.tile_pool(name="xp", bufs=6))
            yp = ctx.enter_context(tc.tile_pool(name="yp", bufs=3))
            hp = ctx.enter_context(tc.tile_pool(name="hp", bufs=3))
            op = ctx.enter_context(tc.tile_pool(name="op", bufs=3))
            ypp = ctx.enter_context(tc.tile_pool(name="ypp", bufs=1, space="PSUM"))
            hpp = ctx.enter_context(tc.tile_pool(name="hpp", bufs=1, space="PSUM"))
            opp = ctx.enter_context(tc.tile_pool(name="opp", bufs=5, space="PSUM"))

            # xT: [c-in-chunk partition, chunk, row]; 1KB runs per partition
            xv = x_d.rearrange("t (k p) n -> t p k n", p=128)
            # out: row l at partition l//4, slot l%4 -> 3KB contiguous runs
            ov = o_d.rearrange("(t p s) o -> t p s o", p=128, s=NS)

            # x(0) first so the big load starts before the small const loads;
            # split in two so the cold descriptor-fetch latency overlaps
            x_tiles = {}
            x_tiles[0] = xp.tile([128, KC, TR], bf16, name="x_sb")
            nc.sync.dma_start(x_tiles[0][:, 0, :], xv[0][:, 0])
            nc.sync.dma_start(x_tiles[0][:, 1, :], xv[0][:, 1])

            r_sb = consts.tile([128, 128], bf16)
            nc.sync.dma_start(r_sb, r_d)
            w1_sb = consts.tile([128, KC, H], bf16)
            nc.sync.dma_start(w1_sb, w1_d.rearrange("k p h -> p k h"))
            bh_sb = consts.tile([H, 1], f32)
            nc.sync.dma_start(bh_sb, bh_d)
            fcw_sb = consts.tile([H + 1, OUTP], bf16)
            nc.sync.dma_start(fcw_sb, fcw_d)

            # PE warm-up: ~5us of dense back-to-back matmul streaming while
            # the first x tile loads, to release the PE clock gate (2.4 GHz
            # p-state needs a sustained high-activity stretch; without it the
            # whole kernel runs at the 1.2 GHz mid p-state).
            wu_sb = consts.tile([128, TR], bf16)
            nc.gpsimd.memset(wu_sb, 0.5)
            wu_ps = opp.tile([128, OUTP], f32, name="o_ps")
            NWU = 14
            for i in range(NWU):
                nc.tensor.matmul(wu_ps, lhsT=wu_sb[:, 0:128],
                                 rhs=wu_sb[:, 0:OUTP],
                                 start=(i == 0), stop=(i == NWU - 1))

            # hT rotates through 3 explicit buffers whose constant ones row
            # (the fc_b fold) is memset ONCE, keeping the per-tile gpsimd
            # memset out of the stage-3 dependency chain.
            hT_bufs = [hp.tile([H + 1, TR], bf16, name=f"hT{i}")
                       for i in range(3)]
            for b in hT_bufs:
                nc.gpsimd.memset(b[H:H + 1, :], 1.0)

            # Software-pipelined with every stage consuming full-iteration-old
            # inputs: iteration t runs mix(t-1), y(t), stage3(t-2) on the
            # tensor engine, so no segment waits on same-iteration copies.
            y_tiles = {}
            if nt > 1:
                x_tiles[1] = xp.tile([128, KC, TR], bf16, name="x_sb")
                nc.sync.dma_start(x_tiles[1], xv[1])
            for t in range(nt + 2):
                if t + 2 < nt:
                    x_tiles[t + 2] = xp.tile([128, KC, TR], bf16,
                                             name="x_sb")
                    nc.sync.dma_start(x_tiles[t + 2], xv[t + 2])

                if 1 <= t <= nt:
                    # mix-stage (tile t-1): hT = y_sub^T @ (I8 (x) lapT)
                    y_sb = y_tiles.pop(t - 1)
                    hT_ps = hpp.tile([H, TR], f32)
                    for s in range(NS):
                        nc.tensor.matmul(
                            hT_ps[:, s * 128:(s + 1) * 128],
                            lhsT=y_sb[:, s, :],
                            rhs=r_sb,
                        )
                    hT_sb = hT_bufs[(t - 1) % 3]
                    nc.scalar.activation(hT_sb[0:H, :], hT_ps, Relu,
                                         bias=bh_sb)

                if t < nt:
                    # y-stage: y_sub[n,h] = sum_k xT[k][:,n-slice]^T @ W1_k
                    x_sb = x_tiles.pop(t)
                    y_ps = ypp.tile([128, NS, H], f32)
                    for s in range(NS):
                        for k in range(KC):
                            nc.tensor.matmul(
                                y_ps[:, s, :],
                                lhsT=x_sb[:, k, s * 128:(s + 1) * 128],
                                rhs=w1_sb[:, k, :],
                                start=(k == 0),
                                stop=(k == KC - 1),
                            )
                    y_sb = yp.tile([128, NS, H], bf16)
                    nc.scalar.copy(y_sb, y_ps)
                    y_tiles[t] = y_sb

                if t >= 2:
                    # stage 3 (tile t-2): slot s covers rows l = NS*p + s.
                    # For the last two tiles (the pipeline epilogue, nothing
                    # left to overlap with) split the copies across both
                    # PSUM-capable engines and store each half as soon as its
                    # copies land, on separate DGE rings, to shorten the tail.
                    last = (t - 2) >= nt - 2
                    hT_sb = hT_bufs[(t - 2) % 3]
                    o_sb = op.tile([128, NS, OUT], bf16)
                    hT_v = hT_sb.rearrange("h (n s) -> h s n", s=NS)
                    for s in range(NS):
                        o_ps = opp.tile([128, OUTP], f32)
                        nc.tensor.matmul(
                            o_ps,
                            lhsT=hT_v[:, s, :],
                            rhs=fcw_sb,
                        )
                        if last and s % 2 == 1:
                            nc.scalar.copy(o_sb[:, s, :], o_ps[:, 0:OUT])
                        else:
                            nc.vector.tensor_copy(o_sb[:, s, :],
                                                  o_ps[:, 0:OUT])
                        if last and s == NS // 2 - 1:
                            nc.sync.dma_start(ov[t - 2][:, 0:NS // 2, :],
                                              o_sb[:, 0:NS // 2, :])
                    if last:
                        nc.scalar.dma_start(ov[t - 2][:, NS // 2:, :],
                                            o_sb[:, NS // 2:, :])
                    else:
                        nc.scalar.dma_start(ov[t - 2], o_sb)

    nc.finalize()
    return nc


def _host_prep(adj, adj_bias, cheb_w, brelu_bias, fc_w, fc_b):
    import ml_dtypes

    bf = ml_dtypes.bfloat16
    adj = np.asarray(adj, np.float32)
    w = np.maximum(adj + np.float32(adj_bias.reshape(())), 0.0)
    d = 1.0 / np.sqrt(w.sum(axis=1) + np.float32(1e-5))
    lap = np.eye(E, dtype=np.float32) - d[:, None] * w * d[None, :]

    # r = I_8 (x) lap^T : [p = b*16+j, n = b*16+i] -> lap[i, j]
    r = np.kron(np.eye(128 // E, dtype=np.float32), lap.T)

    cheb_w = np.asarray(cheb_w, np.float32)
    w1 = np.ascontiguousarray(cheb_w[1::2, :]).reshape(KC, 128, H)
    bias_h = (cheb_w[0::2, :].sum(axis=0)
              + np.asarray(brelu_bias, np.float32).reshape(H))
    fcw = np.zeros((H + 1, OUTP), np.float32)
    fcw[:H, :OUT] = np.asarray(fc_w, np.float32).T
    fcw[H, :OUT] = np.asarray(fc_b, np.float32)
    return {
        "r": r.astype(bf),
        "w1": np.ascontiguousarray(w1).astype(bf),
        "bh": bias_h.reshape(H, 1).astype(np.float32),
        "fcw": fcw.astype(bf),
    }


def _run(inputs, trace=False, nt=NT, **kw):
    import ml_dtypes
    from concourse import bass_utils

    if nt not in _cache:
        _cache[nt] = _build_module(nt)
    nc = _cache[nt]

    # host: cast to bf16 and transpose each 512-row macro tile to [C, 512]
    x = np.asarray(inputs["x"], np.float32).astype(ml_dtypes.bfloat16)
    xt = np.ascontiguousarray(
        x.reshape(NCORES, NT, TR, C)[:, :nt].transpose(0, 1, 3, 2))
    weights = _host_prep(inputs["adj"], inputs["adj_bias"], inputs["cheb_w"],
                         inputs["brelu_bias"], inputs["fc_w"], inputs["fc_b"])

    in_maps = [dict(weights, x=xt[c]) for c in range(NCORES)]

    res = bass_utils.run_bass_kernel_spmd(
        nc, in_maps, core_ids=list(range(NCORES)), trace=trace, **kw)

    out = np.concatenate(
        [res.results[c]["o"].astype(np.float32).reshape(-1, E, OUT)
         for c in range(NCORES)], axis=0)
    return out, res


def kernel(**inputs) -> np.ndarray:
    out, _ = _run(inputs, trace=False)
    return out
